# revision 37
# baseline (speedup 1.0000x reference)
"""Trainium2 Bass kernel for nn_BondOutputModule (gnn_message_passing).

Reference computation:
    hv = h @ W_out                       (projection pulled before segment sum)
    out[t,b] = sum_{e in type t, graph b} hv[src_e]
    graph_v[b,t]; mask; softmax over t

Device strategy (8 cores, SPMD), v10 "fastonehot":
  Owner-compute as v9 (edges live on the core owning their source node;
  zero gathers), but the scatter is restructured around the DVE 2x perf
  mode and a [128 x 72] bin factorization:
  - bins: global bin (t, b) lives at psum[b % 128, 36*(b >> 7) + t].
  - per slot (128 edges): lhsT = onehot128(b % 128), rhs =
    onehot72(36*(b>>7) + t) * hv[src].  psum [128, 72] accumulates all
    slots; bins land directly in [graph-row, (half, type)-col] layout,
    so the tail needs NO transposes.
  - one-hots are built with tensor_tensor is_equal in [p, o, c] layout
    (one-hot index o in the middle, slot c packed last) so every operand
    has a packed 2-byte last dim -> DVE 2x_1p mode (0.52 ns/elem/part).
    The compare target iotaM[p, o, c] = o is a host-uploaded constant.
  - value scaling runs on the Scalar engine: activation(Copy,
    scale=hv[:, col]) per column range - off the DVE critical path.
  - phase 1 (hv = h . W) runs on the PE: host uploads h transposed
    (two 96-deep chunks), 196 tiny matmuls accumulate into psum[128, 98].
  - tail: psum -> AllReduce [9216] -> mask -> softmax (interleaved
    [128, 72] layout, stride-2 slices per graph half).
"""
import sys

if '/opt/trn_rl_repo' not in sys.path:
    sys.path.insert(0, '/opt/trn_rl_repo')

import numpy as np

TRACE = [False]          # test harness can set kernel.TRACE[0] = True
LAST_EXEC_NS = [None]    # filled when TRACE is on

N = 100000
D = 192
T = 36
E = 30000
B = 256
NCORES = 8
NSH = N // NCORES          # 12500 nodes per core
NSHP = 12544               # padded to 98*128
NCOL = NSHP // 128         # 98 node columns
CPB = 32                   # slots per one-hot batch
PAD128 = 200.0             # seg128 pad (no match in 0..127)
PAD72 = 100.0              # code72 pad (no match in 0..71)
DC = 96                    # contraction depth per phase-1 matmul chunk
HBL = 14                   # phase-1 column-blocks per DMA


def _patch_tile_drain():
    """This walrus build accepts at most one sync-wait per CTRL/DMA
    instruction; Tile's tail drain can carry one wait per DMA lane."""
    import concourse.tile as tile
    from concourse.vector_clock import ScopedClock
    from concourse import mybir

    if getattr(tile.TileContext, '_bondout_patched', False):
        return

    def _drain_and_barrier(self, tick_clock, wait_clock):
        nc = self.nc
        carriers = [nc.sync.nop(nofuse=True, hint=f"dw{i}") for i in range(24)]
        drain_inst = nc.sync.drain()
        wait_clock.add_sem_waits(
            drain_inst.ins, ScopedClock({None: tick_clock.global_clock})
        )
        waits = list(drain_inst.ins.sync_info.on_wait)
        if len(waits) > 1:
            drain_inst.ins.sync_info.on_wait = waits[-1:]
            for c, w in zip(carriers, waits[:-1]):
                if c.ins.sync_info is None:
                    c.ins.sync_info = mybir.SyncInfo(on_wait=[w], on_update=[])
                else:
                    c.ins.sync_info.on_wait = [w]
        nc.all_engine_barrier()
        assert self.sems is not None
        popped = nc._tile_sem_poison_stack.pop()
        assert popped is self._sem_poison
        nc.clear_and_free_semaphores(list(self.sems.allocated().values()))
        nc.all_engine_barrier()

    tile.TileContext._drain_and_barrier = _drain_and_barrier
    tile.TileContext._bondout_patched = True


def _split_multi_waits(nc):
    from concourse import mybir
    for f in nc.m.functions:
        for blk in f.blocks:
            new = []
            changed = False
            for inst in blk.instructions:
                si = inst.sync_info
                if si is not None and si.on_wait and len(si.on_wait) > 1:
                    waits = list(si.on_wait)
                    for j, w in enumerate(waits[:-1]):
                        nop = mybir.InstNoOp(
                            name=f"{inst.name}-ws{j}",
                            engine=inst.engine,
                            bass_nofuse=True,
                            sync_info=mybir.SyncInfo(on_wait=[w], on_update=[]),
                        )
                        new.append(nop)
                    si.on_wait = waits[-1:]
                    changed = True
                new.append(inst)
            if changed:
                blk.instructions = new
    return nc


def _prepare_edges(edge_src, edge_seg):
    """Per-core node sort by edge count + shared column-K profile + per-slot
    bin-code arrays.

    Returns (Kcols, NCH, per_core) with per_core[k] = dict(
        perm = sigma (node order for h upload),
        s128 = [128, NCH] f32 (b % 128, pad 200),
        s72  = [128, NCH] f32 (2t + b//128, pad 100)).
    """
    src = edge_src.astype(np.int64).reshape(-1)
    typ = np.repeat(np.arange(T, dtype=np.int64), E)
    seg = edge_seg.astype(np.int64).reshape(-1)
    core = src // NSH
    nl = src - core * NSH

    per_core_raw = []
    cnt_sorted = np.zeros((NCORES, NSHP), np.int64)
    for k in range(NCORES):
        m = core == k
        cnt = np.bincount(nl[m], minlength=NSHP)
        sigma = np.argsort(-cnt, kind="stable")          # node order
        cnt_sorted[k] = cnt[sigma]
        per_core_raw.append((m, sigma))

    # shared K profile: per column of 128 sorted nodes, max count over cores
    Kcols = cnt_sorted.reshape(NCORES, NCOL, 128).max(axis=2).max(axis=0)
    Kcols = np.maximum(Kcols, 0)
    NCH = int(Kcols.sum())
    col_start = np.concatenate([[0], np.cumsum(Kcols)])

    per_core = []
    for k in range(NCORES):
        m, sigma = per_core_raw[k]
        spos = np.empty(NSHP, np.int64)
        spos[sigma] = np.arange(NSHP)                    # node -> sigma pos
        sp = spos[nl[m]]                                 # [edges] sigma pos
        p = sp % 128
        i = sp // 128
        # rank of each edge within its node
        order = np.argsort(sp, kind="stable")
        sps = sp[order]
        starts = np.zeros(NSHP, np.int64)
        starts[1:] = np.cumsum(np.bincount(sps, minlength=NSHP))[:-1]
        rank_sorted = np.arange(len(sps)) - starts[sps]
        ksl = np.empty(len(sps), np.int64)
        ksl[order] = rank_sorted
        ch = col_start[i] + ksl
        x = 36 * seg[m] + typ[m]                 # global bin, b-major
        s_r = np.full((128, NCH), PAD128, np.float32)
        s_c = np.full((128, NCH), PAD72, np.float32)
        s_r[p, ch] = x % 96
        s_c[p, ch] = x // 96
        per_core.append({"perm": sigma, "s_r": s_r, "s_c": s_c})
    return Kcols, NCH, per_core


def _build_program(Kcols, NCH):
    import concourse.bass as bass
    from concourse import bacc, mybir
    import concourse.tile as tile

    _patch_tile_drain()
    FP = mybir.dt.float32
    F16 = mybir.dt.float16
    NB = -(-NCH // CPB)
    col_start = np.concatenate([[0], np.cumsum(Kcols)]).astype(int)

    nc = bacc.Bacc(num_swdge_queues=4)
    ht_in = nc.dram_tensor("ht", [DC, 2 * NCOL * 128], F16, kind="ExternalInput")
    w_in = nc.dram_tensor("wt", [DC, 2], F16, kind="ExternalInput")
    sr_in = nc.dram_tensor("s_r", [128, NCH], F16, kind="ExternalInput")
    sc_in = nc.dram_tensor("s_c", [128, NCH], F16, kind="ExternalInput")
    eye_in = nc.dram_tensor("eye96", [96, 96], FP, kind="ExternalInput")
    m0_in = nc.dram_tensor("mask_keep", [128, 72], FP, kind="ExternalInput")
    mn_in = nc.dram_tensor("mask_neg", [128, 72], FP, kind="ExternalInput")
    out_t = nc.dram_tensor("out", [256, 36], FP, kind="ExternalOutput")

    with tile.TileContext(nc) as tc:
        with (tc.tile_pool(name="dram", bufs=1, space="DRAM") as dram,
              tc.tile_pool(name="const", bufs=1) as cp,
              tc.tile_pool(name="hin", bufs=14) as hp,
              tc.tile_pool(name="oh", bufs=6) as ohp,
              tc.tile_pool(name="p1", bufs=1, space="PSUM") as p1p,
              tc.tile_pool(name="p2", bufs=1, space="PSUM") as p2p,
              tc.tile_pool(name="tp", bufs=1, space="PSUM") as tpp,
              tc.tile_pool(name="fin", bufs=1) as fp_pool):
            # preload constants / slot metadata (issued on sync; the hT
            # stream below goes out on gpsimd whose DMA dispatch is cheap)
            wt = cp.tile([DC, 2], F16)
            nc.sync.dma_start(wt[:], w_in[:])
            srt = cp.tile([128, NCH], F16)
            nc.sync.dma_start(srt[:], sr_in[:])
            sct = cp.tile([128, NCH], F16)
            nc.sync.dma_start(sct[:], sc_in[:])

            # phase-1 hT stream: issue ALL loads first from gpsimd (cheap
            # DMA dispatch), before the iota work occupies that engine
            ps1 = p1p.tile([128, NCOL], FP)
            hv32 = cp.tile([128, NCOL], FP)
            nblk = 2 * NCOL                      # (column, dchunk) blocks
            NB1 = -(-nblk // HBL)
            htts = []
            for ld in range(NB1):
                b0 = ld * HBL
                nb = min(HBL, nblk - b0)
                htt = hp.tile([DC, HBL * 128], F16, tag="htt")
                nc.gpsimd.dma_start(
                    htt[:, 0:nb * 128], ht_in[:, b0 * 128:(b0 + nb) * 128])
                htts.append((htt, b0, nb))

            # iota compare target built on the (otherwise idle) Pool engine
            # (shared by both one-hot sides: both are 96 wide)
            im96 = cp.tile([128, 96 * CPB], F16)
            im96v = im96[:].rearrange("p (o c) -> p o c", c=CPB)
            nc.gpsimd.iota(im96v, pattern=[[1, 96], [0, CPB]], base=0,
                           channel_multiplier=0,
                           allow_small_or_imprecise_dtypes=True)

            # warm up the collective stream with a tiny AllReduce so the
            # ring setup cost overlaps phase 1 instead of the tail
            wz = fp_pool.tile([1, 16], FP, tag="wz")
            nc.gpsimd.memset(wz[:], 0.0)
            wu_in = dram.tile([16], FP, tag="wui")
            nc.sync.dma_start(wu_in[:].rearrange("(p j) -> p j", p=1), wz[:])
            wu_out = dram.tile([16], FP, tag="wuo")
            nc.gpsimd.collective_compute(
                "AllReduce", mybir.AluOpType.add,
                replica_groups=[list(range(NCORES))],
                ins=[wu_in.opt()], outs=[wu_out.opt()])

            # ---------- phase 1 (PE): hv[p, i] = h[128i+p] . W ----------
            # matmuls issued interleaved with phase 2 chunks below (PE
            # executes in order - phase 2 work must not sit behind
            # DMA-starved phase 1 matmuls).
            ph1_done = [0]

            def issue_ph1(n):
                for ld in range(ph1_done[0], min(n, NB1)):
                    htt, b0, nb = htts[ld]
                    for jj in range(nb):
                        blk = b0 + jj
                        i, k = blk // 2, blk % 2
                        nc.tensor.matmul(
                            out=ps1[:, i:i + 1],
                            lhsT=htt[:, jj * 128:(jj + 1) * 128],
                            rhs=wt[:, k:k + 1],
                            start=(k == 0), stop=(k == 1),
                            skip_group_check=True)
                    c_lo = b0 // 2 + (b0 % 2 > 0)
                    c_hi = (b0 + nb) // 2
                    if c_hi > c_lo:
                        nc.scalar.copy(out=hv32[:, c_lo:c_hi],
                                       in_=ps1[:, c_lo:c_hi])
                ph1_done[0] = max(ph1_done[0], min(n, NB1))

            # ---------- phase 2: one-hot matmul scatter ----------
            # two interleaved psum accumulation chains (even/odd slots) so
            # ldweights of one chain overlaps matmul of the other
            ps2a = p2p.tile([96, 96], FP, tag="ps2a")
            ps2b = p2p.tile([96, 96], FP, tag="ps2b")
            mulct = [0]
            cols_per_batch = HBL // 2
            for bb in range(NB):
                c0, c1 = bb * CPB, min((bb + 1) * CPB, NCH)
                nn = c1 - c0
                need_col = int(np.searchsorted(col_start, c1, side="left"))
                need = max(-(-(need_col + cols_per_batch) // cols_per_batch),
                           min(bb + 1, NB1))
                issue_ph1(need)
                ohc = ohp.tile([128, 96 * CPB], F16, tag="ohc")
                ocv = ohc[:].rearrange("p (o c) -> p o c", c=CPB)
                nc.vector.tensor_tensor(
                    out=ocv[:, :, 0:nn],
                    in0=sct[:, c0:c1].rearrange("p (o c) -> p o c", o=1)
                        .to_broadcast([128, 96, nn]),
                    in1=im96v[:, :, 0:nn],
                    op=mybir.AluOpType.is_equal)
                ohr = ohp.tile([128, 96 * CPB], F16, tag="ohr")
                orv = ohr[:].rearrange("p (o c) -> p o c", c=CPB)
                nc.vector.tensor_tensor(
                    out=orv[:, :, 0:nn],
                    in0=srt[:, c0:c1].rearrange("p (o c) -> p o c", o=1)
                        .to_broadcast([128, 96, nn]),
                    in1=im96v[:, :, 0:nn],
                    op=mybir.AluOpType.is_equal)
                # value scaling, alternating Scalar / Vector(ts 4x) engines
                ohv = ohp.tile([128, 96 * CPB], F16, tag="ohv")
                ov = ohv[:].rearrange("p (o c) -> p o c", c=CPB)
                ic0 = int(np.searchsorted(col_start, c0, side="right")) - 1
                ic1 = int(np.searchsorted(col_start, c1, side="left"))
                for i in range(ic0, ic1):
                    a = max(c0, int(col_start[i]))
                    b_ = min(c1, int(col_start[i + 1]))
                    if a >= b_:
                        continue
                    with nc.allow_low_precision(reason="fp16 edge vals"):
                        if mulct[0] % 5 < 3:
                            nc.scalar.activation(
                                out=ov[:, :, a - c0:b_ - c0],
                                in_=ocv[:, :, a - c0:b_ - c0],
                                func=mybir.ActivationFunctionType.Copy,
                                scale=hv32[:, i:i + 1])
                        else:
                            nc.vector.tensor_scalar(
                                out=ov[:, :, a - c0:b_ - c0],
                                in0=ocv[:, :, a - c0:b_ - c0],
                                scalar1=hv32[:, i:i + 1], scalar2=None,
                                op0=mybir.AluOpType.mult)
                    mulct[0] += 1
                for c in range(c0, c1):
                    ps = ps2a if (c % 2 == 0) else ps2b
                    nc.tensor.matmul(
                        out=ps[:, 0:96],
                        lhsT=orv[:, :, c - c0:c - c0 + 1],
                        rhs=ov[:, :, c - c0:c - c0 + 1],
                        start=(c < 2), stop=(c >= NCH - 2),
                        skip_group_check=True)

            issue_ph1(NB1)

            # ---------- AllReduce + mask + softmax ----------
            sba = fp_pool.tile([96, 96], FP, tag="sba")
            nc.vector.tensor_copy(out=sba[:], in_=ps2a[:])
            sb = fp_pool.tile([96, 96], FP, tag="sb")
            nc.vector.tensor_tensor(
                out=sb[:], in0=sba[:], in1=ps2b[:], op=mybir.AluOpType.add)
            # transpose so DRAM holds bins in flat x = 36*b + t order -
            # the post-reduce reload is then a clean affine AP
            eye = cp.tile([96, 96], FP)
            nc.sync.dma_start(eye[:], eye_in[:])
            tp = tpp.tile([96, 96], FP, tag="tp")
            nc.tensor.transpose(out=tp[:], in_=sb[:], identity=eye[:])
            sbT = fp_pool.tile([96, 96], FP, tag="sbT")
            nc.vector.tensor_copy(out=sbT[:], in_=tp[:])
            part_d = dram.tile([96 * 96], FP, tag="part")
            nc.sync.dma_start(
                part_d[:].rearrange("(q j) -> q j", q=96), sbT[:])
            red_d = dram.tile([96 * 96], FP, tag="red")
            nc.gpsimd.collective_compute(
                "AllReduce", mybir.AluOpType.add,
                replica_groups=[list(range(NCORES))],
                ins=[part_d.opt()], outs=[red_d.opt()])
            a_sb = fp_pool.tile([128, 72], FP, tag="asb")
            nc.sync.dma_start(
                a_sb[:].rearrange("p (g t) -> p g t", t=36),
                red_d[:].rearrange("(g p t) -> p g t", g=2, t=36))
            m0 = cp.tile([128, 72], FP)
            nc.sync.dma_start(m0[:], m0_in[:])
            mn = cp.tile([128, 72], FP)
            nc.sync.dma_start(mn[:], mn_in[:])
            gv = fp_pool.tile([128, 72], FP, tag="gv")
            nc.vector.tensor_tensor(
                out=gv[:], in0=a_sb[:], in1=m0[:], op=mybir.AluOpType.mult)
            nc.vector.tensor_tensor(
                out=gv[:], in0=gv[:], in1=mn[:], op=mybir.AluOpType.add)
            for g in range(2):
                mx = fp_pool.tile([128, 1], FP, tag="mx")
                nc.vector.tensor_reduce(
                    out=mx[:], in_=gv[:, 36 * g:36 * (g + 1)],
                    axis=mybir.AxisListType.X, op=mybir.AluOpType.max)
                gvs = fp_pool.tile([128, 36], FP, tag="gvs")
                nc.vector.tensor_scalar(
                    out=gvs[:], in0=gv[:, 36 * g:36 * (g + 1)],
                    scalar1=mx[:], scalar2=None,
                    op0=mybir.AluOpType.subtract)
                ex = fp_pool.tile([128, 36], FP, tag="ex")
                sm = fp_pool.tile([128, 1], FP, tag="sm")
                nc.scalar.activation(
                    out=ex[:], in_=gvs[:],
                    func=mybir.ActivationFunctionType.Exp,
                    accum_out=sm[:])
                rec = fp_pool.tile([128, 1], FP, tag="rec")
                nc.vector.reciprocal(rec[:], sm[:])
                res = fp_pool.tile([128, 36], FP, tag="res")
                nc.vector.tensor_scalar(
                    out=res[:], in0=ex[:], scalar1=rec[:], scalar2=None,
                    op0=mybir.AluOpType.mult)
                nc.sync.dma_start(out_t[g * 128:(g + 1) * 128, :], res[:])

    nc.move_matmul_waits_to_ldweights()
    nc.compile()
    _split_multi_waits(nc)
    return nc


def kernel(h, W_out, edge_src, edge_seg, mask_mat):
    from concourse.bass_utils import run_bass_kernel_spmd

    h = np.ascontiguousarray(h, np.float32)
    W_out = np.ascontiguousarray(W_out, np.float32)
    Kcols, NCH, per_core = _prepare_edges(edge_src, edge_seg)

    wt = np.zeros((DC, 2), np.float16)
    wt[:, 0] = W_out[0:DC, 0]
    wt[0:D - DC, 1] = W_out[DC:D, 0]

    # masks in the bin layout: [p, 36g + t] for graph g*128+p
    mk = np.zeros((128, 72), np.float32)
    mn = np.zeros((128, 72), np.float32)
    for g in range(2):
        mm = mask_mat[g * 128:(g + 1) * 128, :]       # [128, 36] bool
        mk[:, 36 * g:36 * (g + 1)] = (~mm).astype(np.float32)
        mn[:, 36 * g:36 * (g + 1)] = mm.astype(np.float32) * np.float32(-1e9)

    in_maps = []
    for k in range(NCORES):
        hs = np.zeros((NSHP, D), np.float32)
        hs[:NSH] = h[k * NSH:(k + 1) * NSH]
        hperm = hs[per_core[k]["perm"]].astype(np.float16)   # [NSHP, D]
        # hT blocks: ht[p, (2i+k)*128 + m] = hperm[128i + m, DC*k + p]
        hT = np.ascontiguousarray(hperm.reshape(NCOL, 128, 2, DC)
                                  .transpose(3, 0, 2, 1)
                                  .reshape(DC, 2 * NCOL * 128))
        in_maps.append({
            "ht": hT,
            "wt": wt,
            "s_r": per_core[k]["s_r"].astype(np.float16),
            "s_c": per_core[k]["s_c"].astype(np.float16),
            "eye96": np.eye(96, dtype=np.float32),
            "mask_keep": mk,
            "mask_neg": mn,
        })

    nc = _build_program(Kcols, NCH)
    kwargs = {}
    if TRACE[0]:
        import tempfile
        kwargs = dict(trace=True, tmpdir=tempfile.mkdtemp(prefix="bondout_"))
    res = run_bass_kernel_spmd(nc, in_maps, core_ids=list(range(NCORES)),
                               **kwargs)
    LAST_EXEC_NS[0] = res.exec_time_ns
    return np.asarray(res.results[0]["out"], np.float32)


# revision 46
# speedup vs baseline: 1.1573x; 1.1573x over previous
"""Trainium2 Bass kernel for nn_BondOutputModule (gnn_message_passing).

Reference computation:
    hv = h @ W_out                       (projection pulled before segment sum)
    out[t,b] = sum_{e in type t, graph b} hv[src_e]
    graph_v[b,t]; mask; softmax over t

Device strategy (8 cores, SPMD), v10 "fastonehot":
  Owner-compute as v9 (edges live on the core owning their source node;
  zero gathers), but the scatter is restructured around the DVE 2x perf
  mode and a [128 x 72] bin factorization:
  - bins: global bin (t, b) lives at psum[b % 128, 36*(b >> 7) + t].
  - per slot (128 edges): lhsT = onehot128(b % 128), rhs =
    onehot72(36*(b>>7) + t) * hv[src].  psum [128, 72] accumulates all
    slots; bins land directly in [graph-row, (half, type)-col] layout,
    so the tail needs NO transposes.
  - one-hots are built with tensor_tensor is_equal in [p, o, c] layout
    (one-hot index o in the middle, slot c packed last) so every operand
    has a packed 2-byte last dim -> DVE 2x_1p mode (0.52 ns/elem/part).
    The compare target iotaM[p, o, c] = o is a host-uploaded constant.
  - value scaling runs on the Scalar engine: activation(Copy,
    scale=hv[:, col]) per column range - off the DVE critical path.
  - phase 1 (hv = h . W) runs on the PE: host uploads h transposed
    (two 96-deep chunks), 196 tiny matmuls accumulate into psum[128, 98].
  - tail: psum -> AllReduce [9216] -> mask -> softmax (interleaved
    [128, 72] layout, stride-2 slices per graph half).
"""
import sys

if '/opt/trn_rl_repo' not in sys.path:
    sys.path.insert(0, '/opt/trn_rl_repo')

import numpy as np

TRACE = [False]          # test harness can set kernel.TRACE[0] = True
LAST_EXEC_NS = [None]    # filled when TRACE is on

N = 100000
D = 192
T = 36
E = 30000
B = 256
NCORES = 8
NSH = N // NCORES          # 12500 nodes per core
NSHP = 12544               # padded to 98*128
NCOL = NSHP // 128         # 98 node columns
CPB = 32                   # slots per one-hot batch
PAD128 = 200.0             # seg128 pad (no match in 0..127)
PAD72 = 100.0              # code72 pad (no match in 0..71)
DC = 96                    # contraction depth per phase-1 matmul chunk
HBL = 14                   # phase-1 column-blocks per DMA


def _patch_tile_drain():
    """This walrus build accepts at most one sync-wait per CTRL/DMA
    instruction; Tile's tail drain can carry one wait per DMA lane."""
    import concourse.tile as tile
    from concourse.vector_clock import ScopedClock
    from concourse import mybir

    if getattr(tile.TileContext, '_bondout_patched', False):
        return

    def _drain_and_barrier(self, tick_clock, wait_clock):
        nc = self.nc
        carriers = [nc.sync.nop(nofuse=True, hint=f"dw{i}") for i in range(24)]
        drain_inst = nc.sync.drain()
        wait_clock.add_sem_waits(
            drain_inst.ins, ScopedClock({None: tick_clock.global_clock})
        )
        waits = list(drain_inst.ins.sync_info.on_wait)
        if len(waits) > 1:
            drain_inst.ins.sync_info.on_wait = waits[-1:]
            for c, w in zip(carriers, waits[:-1]):
                if c.ins.sync_info is None:
                    c.ins.sync_info = mybir.SyncInfo(on_wait=[w], on_update=[])
                else:
                    c.ins.sync_info.on_wait = [w]
        nc.all_engine_barrier()
        assert self.sems is not None
        popped = nc._tile_sem_poison_stack.pop()
        assert popped is self._sem_poison
        nc.clear_and_free_semaphores(list(self.sems.allocated().values()))
        nc.all_engine_barrier()

    tile.TileContext._drain_and_barrier = _drain_and_barrier
    tile.TileContext._bondout_patched = True


def _split_multi_waits(nc):
    from concourse import mybir
    for f in nc.m.functions:
        for blk in f.blocks:
            new = []
            changed = False
            for inst in blk.instructions:
                si = inst.sync_info
                if si is not None and si.on_wait and len(si.on_wait) > 1:
                    waits = list(si.on_wait)
                    for j, w in enumerate(waits[:-1]):
                        nop = mybir.InstNoOp(
                            name=f"{inst.name}-ws{j}",
                            engine=inst.engine,
                            bass_nofuse=True,
                            sync_info=mybir.SyncInfo(on_wait=[w], on_update=[]),
                        )
                        new.append(nop)
                    si.on_wait = waits[-1:]
                    changed = True
                new.append(inst)
            if changed:
                blk.instructions = new
    return nc


def _prepare_edges(edge_src, edge_seg):
    """Per-core node sort by edge count + shared column-K profile + per-slot
    bin-code arrays.

    Returns (Kcols, NCH, per_core) with per_core[k] = dict(
        perm = sigma (node order for h upload),
        s128 = [128, NCH] f32 (b % 128, pad 200),
        s72  = [128, NCH] f32 (2t + b//128, pad 100)).
    """
    src = edge_src.astype(np.int64).reshape(-1)
    typ = np.repeat(np.arange(T, dtype=np.int64), E)
    seg = edge_seg.astype(np.int64).reshape(-1)
    core = src // NSH
    nl = src - core * NSH

    per_core_raw = []
    cnt_sorted = np.zeros((NCORES, NSHP), np.int64)
    for k in range(NCORES):
        m = core == k
        cnt = np.bincount(nl[m], minlength=NSHP)
        sigma = np.argsort(-cnt, kind="stable")          # node order
        cnt_sorted[k] = cnt[sigma]
        per_core_raw.append((m, sigma))

    # shared K profile: per column of 128 sorted nodes, max count over cores
    Kcols = cnt_sorted.reshape(NCORES, NCOL, 128).max(axis=2).max(axis=0)
    Kcols = np.maximum(Kcols, 0)
    NCH = int(Kcols.sum())
    col_start = np.concatenate([[0], np.cumsum(Kcols)])

    per_core = []
    for k in range(NCORES):
        m, sigma = per_core_raw[k]
        spos = np.empty(NSHP, np.int64)
        spos[sigma] = np.arange(NSHP)                    # node -> sigma pos
        sp = spos[nl[m]]                                 # [edges] sigma pos
        p = sp % 128
        i = sp // 128
        # rank of each edge within its node
        order = np.argsort(sp, kind="stable")
        sps = sp[order]
        starts = np.zeros(NSHP, np.int64)
        starts[1:] = np.cumsum(np.bincount(sps, minlength=NSHP))[:-1]
        rank_sorted = np.arange(len(sps)) - starts[sps]
        ksl = np.empty(len(sps), np.int64)
        ksl[order] = rank_sorted
        ch = col_start[i] + ksl
        s_r = np.full((128, NCH), PAD128, np.float32)
        s_c = np.full((128, NCH), PAD72, np.float32)
        s_r[p, ch] = seg[m] % 128
        s_c[p, ch] = 36 * (seg[m] // 128) + typ[m]
        per_core.append({"perm": sigma, "s_r": s_r, "s_c": s_c})
    return Kcols, NCH, per_core


def _build_program(Kcols, NCH):
    import concourse.bass as bass
    from concourse import bacc, mybir
    import concourse.tile as tile

    _patch_tile_drain()
    FP = mybir.dt.float32
    F16 = mybir.dt.float16
    NB = -(-NCH // CPB)
    col_start = np.concatenate([[0], np.cumsum(Kcols)]).astype(int)

    nc = bacc.Bacc(num_swdge_queues=4)
    ht_in = nc.dram_tensor("ht", [DC, 2 * NCOL * 128], F16, kind="ExternalInput")
    w_in = nc.dram_tensor("wt", [DC, 2], F16, kind="ExternalInput")
    sr_in = nc.dram_tensor("s_r", [128, NCH], F16, kind="ExternalInput")
    sc_in = nc.dram_tensor("s_c", [128, NCH], F16, kind="ExternalInput")
    m0_in = nc.dram_tensor("mask_keep", [128, 72], FP, kind="ExternalInput")
    mn_in = nc.dram_tensor("mask_neg", [128, 72], FP, kind="ExternalInput")
    out_t = nc.dram_tensor("out", [256, 36], FP, kind="ExternalOutput")

    with tile.TileContext(nc) as tc:
        with (tc.tile_pool(name="dram", bufs=1, space="DRAM") as dram,
              tc.tile_pool(name="const", bufs=1) as cp,
              tc.tile_pool(name="hin", bufs=14) as hp,
              tc.tile_pool(name="oh", bufs=6) as ohp,
              tc.tile_pool(name="p1", bufs=1, space="PSUM") as p1p,
              tc.tile_pool(name="p2", bufs=1, space="PSUM") as p2p,
              tc.tile_pool(name="tp", bufs=1, space="PSUM") as tpp,
              tc.tile_pool(name="fin", bufs=1) as fp_pool):
            # preload constants / slot metadata (issued on sync; the hT
            # stream below goes out on gpsimd whose DMA dispatch is cheap)
            wt = cp.tile([DC, 2], F16)
            nc.sync.dma_start(wt[:], w_in[:])
            srt = cp.tile([128, NCH], F16)
            nc.sync.dma_start(srt[:], sr_in[:])
            sct = cp.tile([128, NCH], F16)
            nc.sync.dma_start(sct[:], sc_in[:])

            # phase-1 hT stream: issue ALL loads first from gpsimd (cheap
            # DMA dispatch), before the iota work occupies that engine
            ps1 = p1p.tile([128, NCOL], FP)
            hv32 = cp.tile([128, NCOL], FP)
            nblk = 2 * NCOL                      # (column, dchunk) blocks
            NB1 = -(-nblk // HBL)
            htts = []
            for ld in range(NB1):
                b0 = ld * HBL
                nb = min(HBL, nblk - b0)
                htt = hp.tile([DC, HBL * 128], F16, tag="htt")
                nc.gpsimd.dma_start(
                    htt[:, 0:nb * 128], ht_in[:, b0 * 128:(b0 + nb) * 128])
                htts.append((htt, b0, nb))

            # iota compare targets built on the (otherwise idle) Pool engine
            im72 = cp.tile([128, 72 * CPB], F16)
            im72v = im72[:].rearrange("p (o c) -> p o c", c=CPB)
            nc.gpsimd.iota(im72v, pattern=[[1, 72], [0, CPB]], base=0,
                           channel_multiplier=0,
                           allow_small_or_imprecise_dtypes=True)
            im128 = cp.tile([128, 128 * CPB], F16)
            im128v = im128[:].rearrange("p (o c) -> p o c", c=CPB)
            nc.gpsimd.iota(im128v, pattern=[[1, 128], [0, CPB]], base=0,
                           channel_multiplier=0,
                           allow_small_or_imprecise_dtypes=True)

            # warm up the collective stream with a tiny AllReduce so the
            # ring setup cost overlaps phase 1 instead of the tail
            wz = fp_pool.tile([1, 16], FP, tag="wz")
            nc.gpsimd.memset(wz[:], 0.0)
            wu_in = dram.tile([16], FP, tag="wui")
            nc.sync.dma_start(wu_in[:].rearrange("(p j) -> p j", p=1), wz[:])
            wu_out = dram.tile([16], FP, tag="wuo")
            nc.gpsimd.collective_compute(
                "AllReduce", mybir.AluOpType.add,
                replica_groups=[list(range(NCORES))],
                ins=[wu_in.opt()], outs=[wu_out.opt()])

            # ---------- phase 1 (PE): hv[p, i] = h[128i+p] . W ----------
            # matmuls issued interleaved with phase 2 chunks below (PE
            # executes in order - phase 2 work must not sit behind
            # DMA-starved phase 1 matmuls).
            ph1_done = [0]

            def issue_ph1(n):
                for ld in range(ph1_done[0], min(n, NB1)):
                    htt, b0, nb = htts[ld]
                    for jj in range(nb):
                        blk = b0 + jj
                        i, k = blk // 2, blk % 2
                        nc.tensor.matmul(
                            out=ps1[:, i:i + 1],
                            lhsT=htt[:, jj * 128:(jj + 1) * 128],
                            rhs=wt[:, k:k + 1],
                            start=(k == 0), stop=(k == 1),
                            skip_group_check=True)
                    c_lo = b0 // 2 + (b0 % 2 > 0)
                    c_hi = (b0 + nb) // 2
                    if c_hi > c_lo:
                        nc.scalar.copy(out=hv32[:, c_lo:c_hi],
                                       in_=ps1[:, c_lo:c_hi])
                ph1_done[0] = max(ph1_done[0], min(n, NB1))

            # ---------- phase 2: one-hot matmul scatter ----------
            # two interleaved psum accumulation chains (even/odd slots) so
            # ldweights of one chain overlaps matmul of the other
            ps2a = p2p.tile([128, 72], FP, tag="ps2a")
            ps2b = p2p.tile([128, 72], FP, tag="ps2b")
            ps2c = p2p.tile([128, 72], FP, tag="ps2c")
            ps2d = p2p.tile([128, 72], FP, tag="ps2d")
            pss = [ps2a, ps2b, ps2c, ps2d]
            mulct = [0]
            cols_per_batch = HBL // 2
            for bb in range(NB):
                c0, c1 = bb * CPB, min((bb + 1) * CPB, NCH)
                nn = c1 - c0
                need_col = int(np.searchsorted(col_start, c1, side="left"))
                need = max(-(-(need_col + cols_per_batch) // cols_per_batch),
                           min(bb + 1, NB1))
                issue_ph1(need)
                ohc = ohp.tile([128, 72 * CPB], F16, tag="ohc")
                ocv = ohc[:].rearrange("p (o c) -> p o c", c=CPB)
                nc.vector.tensor_tensor(
                    out=ocv[:, :, 0:nn],
                    in0=sct[:, c0:c1].rearrange("p (o c) -> p o c", o=1)
                        .to_broadcast([128, 72, nn]),
                    in1=im72v[:, :, 0:nn],
                    op=mybir.AluOpType.is_equal)
                ohr = ohp.tile([128, 128 * CPB], F16, tag="ohr")
                orv = ohr[:].rearrange("p (o c) -> p o c", c=CPB)
                nc.vector.tensor_tensor(
                    out=orv[:, :, 0:nn],
                    in0=srt[:, c0:c1].rearrange("p (o c) -> p o c", o=1)
                        .to_broadcast([128, 128, nn]),
                    in1=im128v[:, :, 0:nn],
                    op=mybir.AluOpType.is_equal)
                # value scaling, alternating Scalar / Vector(ts 4x) engines
                ohv = ohp.tile([128, 72 * CPB], F16, tag="ohv")
                ov = ohv[:].rearrange("p (o c) -> p o c", c=CPB)
                ic0 = int(np.searchsorted(col_start, c0, side="right")) - 1
                ic1 = int(np.searchsorted(col_start, c1, side="left"))
                for i in range(ic0, ic1):
                    a = max(c0, int(col_start[i]))
                    b_ = min(c1, int(col_start[i + 1]))
                    if a >= b_:
                        continue
                    with nc.allow_low_precision(reason="fp16 edge vals"):
                        if mulct[0] % 5 < 3:
                            nc.scalar.activation(
                                out=ov[:, :, a - c0:b_ - c0],
                                in_=ocv[:, :, a - c0:b_ - c0],
                                func=mybir.ActivationFunctionType.Copy,
                                scale=hv32[:, i:i + 1])
                        else:
                            nc.vector.tensor_scalar(
                                out=ov[:, :, a - c0:b_ - c0],
                                in0=ocv[:, :, a - c0:b_ - c0],
                                scalar1=hv32[:, i:i + 1], scalar2=None,
                                op0=mybir.AluOpType.mult)
                    mulct[0] += 1
                for c in range(c0, c1):
                    nc.tensor.matmul(
                        out=pss[c % 4][:, 0:72],
                        lhsT=orv[:, :, c - c0:c - c0 + 1],
                        rhs=ov[:, :, c - c0:c - c0 + 1],
                        start=(c < 4), stop=(c >= NCH - 4),
                        skip_group_check=True)

            issue_ph1(NB1)

            # ---------- AllReduce + mask + softmax ----------
            sb = fp_pool.tile([128, 72], FP, tag="sb")
            nc.vector.tensor_copy(out=sb[:], in_=pss[0][:])
            for q in range(1, 4):
                nc.vector.tensor_tensor(
                    out=sb[:], in0=sb[:], in1=pss[q][:],
                    op=mybir.AluOpType.add)
            part_d = dram.tile([128 * 72], FP, tag="part")
            nc.sync.dma_start(
                part_d[:].rearrange("(p j) -> p j", p=128), sb[:])
            red_d = dram.tile([128 * 72], FP, tag="red")
            nc.gpsimd.collective_compute(
                "AllReduce", mybir.AluOpType.add,
                replica_groups=[list(range(NCORES))],
                ins=[part_d.opt()], outs=[red_d.opt()])
            a_sb = fp_pool.tile([128, 72], FP, tag="asb")
            nc.sync.dma_start(
                a_sb[:], red_d[:].rearrange("(p j) -> p j", p=128))
            m0 = cp.tile([128, 72], FP)
            nc.sync.dma_start(m0[:], m0_in[:])
            mn = cp.tile([128, 72], FP)
            nc.sync.dma_start(mn[:], mn_in[:])
            gv = fp_pool.tile([128, 72], FP, tag="gv")
            nc.vector.tensor_tensor(
                out=gv[:], in0=a_sb[:], in1=m0[:], op=mybir.AluOpType.mult)
            nc.vector.tensor_tensor(
                out=gv[:], in0=gv[:], in1=mn[:], op=mybir.AluOpType.add)
            for g in range(2):
                mx = fp_pool.tile([128, 1], FP, tag="mx")
                nc.vector.tensor_reduce(
                    out=mx[:], in_=gv[:, 36 * g:36 * (g + 1)],
                    axis=mybir.AxisListType.X, op=mybir.AluOpType.max)
                gvs = fp_pool.tile([128, 36], FP, tag="gvs")
                nc.vector.tensor_scalar(
                    out=gvs[:], in0=gv[:, 36 * g:36 * (g + 1)],
                    scalar1=mx[:], scalar2=None,
                    op0=mybir.AluOpType.subtract)
                ex = fp_pool.tile([128, 36], FP, tag="ex")
                sm = fp_pool.tile([128, 1], FP, tag="sm")
                nc.scalar.activation(
                    out=ex[:], in_=gvs[:],
                    func=mybir.ActivationFunctionType.Exp,
                    accum_out=sm[:])
                rec = fp_pool.tile([128, 1], FP, tag="rec")
                nc.vector.reciprocal(rec[:], sm[:])
                res = fp_pool.tile([128, 36], FP, tag="res")
                nc.vector.tensor_scalar(
                    out=res[:], in0=ex[:], scalar1=rec[:], scalar2=None,
                    op0=mybir.AluOpType.mult)
                nc.sync.dma_start(out_t[g * 128:(g + 1) * 128, :], res[:])

    nc.move_matmul_waits_to_ldweights()
    nc.compile()
    _split_multi_waits(nc)
    return nc


def kernel(h, W_out, edge_src, edge_seg, mask_mat):
    from concourse.bass_utils import run_bass_kernel_spmd

    h = np.ascontiguousarray(h, np.float32)
    W_out = np.ascontiguousarray(W_out, np.float32)
    Kcols, NCH, per_core = _prepare_edges(edge_src, edge_seg)

    wt = np.zeros((DC, 2), np.float16)
    wt[:, 0] = W_out[0:DC, 0]
    wt[0:D - DC, 1] = W_out[DC:D, 0]

    # masks in the bin layout: [p, 36g + t] for graph g*128+p
    mk = np.zeros((128, 72), np.float32)
    mn = np.zeros((128, 72), np.float32)
    for g in range(2):
        mm = mask_mat[g * 128:(g + 1) * 128, :]       # [128, 36] bool
        mk[:, 36 * g:36 * (g + 1)] = (~mm).astype(np.float32)
        mn[:, 36 * g:36 * (g + 1)] = mm.astype(np.float32) * np.float32(-1e9)

    in_maps = []
    for k in range(NCORES):
        hs = np.zeros((NSHP, D), np.float32)
        hs[:NSH] = h[k * NSH:(k + 1) * NSH]
        hperm = hs[per_core[k]["perm"]].astype(np.float16)   # [NSHP, D]
        # hT blocks: ht[p, (2i+k)*128 + m] = hperm[128i + m, DC*k + p]
        hT = np.ascontiguousarray(hperm.reshape(NCOL, 128, 2, DC)
                                  .transpose(3, 0, 2, 1)
                                  .reshape(DC, 2 * NCOL * 128))
        in_maps.append({
            "ht": hT,
            "wt": wt,
            "s_r": per_core[k]["s_r"].astype(np.float16),
            "s_c": per_core[k]["s_c"].astype(np.float16),
            "mask_keep": mk,
            "mask_neg": mn,
        })

    nc = _build_program(Kcols, NCH)
    kwargs = {}
    if TRACE[0]:
        import tempfile
        kwargs = dict(trace=True, tmpdir=tempfile.mkdtemp(prefix="bondout_"))
    res = run_bass_kernel_spmd(nc, in_maps, core_ids=list(range(NCORES)),
                               **kwargs)
    LAST_EXEC_NS[0] = res.exec_time_ns
    return np.asarray(res.results[0]["out"], np.float32)
